# revision 1
# baseline (speedup 1.0000x reference)
"""Causal attention kernel for Trainium2, 8 NeuronCores, sequence-parallel.

Reference computation (T=4096, D=1024, fp32):
    q = x @ Wqk; logits = q @ x.T (causal masked); attn = softmax(logits)
    out = (attn @ x) @ Wov

Sharding: query rows split 512/core across 8 cores; Wqk/Wov replicated;
each core sees all keys (full x) and computes its row block end-to-end.

Per-core key blocks are permuted host-side so the causal structure is
core-independent: slot 0 = the diagonal 512-block (local triangular mask,
generated on device), slots 1..7 = the remaining blocks, with a per-core
additive bias beta in {0, -1e30} marking fully-visible / fully-masked
blocks. This keeps one SPMD program valid for every core.

Matmul precision: float32r (fp32 with 11-bit mantissa, exact fp32
accumulation) for q/scores/AV/Wov matmuls; softmax row max subtracted in
fp32; attn stored bf16 for the DMA-xbar transposes and AV matmul.
"""

import sys

sys.path.insert(0, "/opt/trn_rl_repo")

import numpy as np
import ml_dtypes

import concourse.tile as tile
from concourse import bacc, mybir
from concourse.bass_utils import run_bass_kernel_spmd

T = 4096
D = 1024
NCORES = 8
RQ = T // NCORES  # 512 query rows per core
NKB = T // 512  # 8 key slots of 512
KC = D // 128  # 8 contraction chunks
NMT = RQ // 128  # 4 query-row tiles per core
NEG = -1.0e30

f32 = mybir.dt.float32
f32r = mybir.dt.float32r
bf16 = mybir.dt.bfloat16


def _round_f32r(a: np.ndarray) -> np.ndarray:
    """Round fp32 to f32r encoding: RNE to 11 explicit mantissa bits."""
    u = np.ascontiguousarray(a, np.float32).view(np.uint32).astype(np.uint64)
    u = (u + 0x7FF + ((u >> 12) & 1)) & ~np.uint64(0xFFF)
    return u.astype(np.uint32).view(np.float32)


def _build_nc():
    nc = bacc.Bacc(
        "TRN2", target_bir_lowering=False, debug=False, num_devices=NCORES
    )

    xqt_d = nc.dram_tensor("xqt", [D, RQ], f32r, kind="ExternalInput").ap()
    xtp_d = nc.dram_tensor("xtp", [D, T], f32r, kind="ExternalInput").ap()
    xp_d = nc.dram_tensor("xp", [T, D], bf16, kind="ExternalInput").ap()
    wqk_d = nc.dram_tensor("wqk", [D, D], f32r, kind="ExternalInput").ap()
    wov_d = nc.dram_tensor("wov", [D, D], f32r, kind="ExternalInput").ap()
    beta_d = nc.dram_tensor("beta", [128, NKB], f32, kind="ExternalInput").ap()
    out_d = nc.dram_tensor("out", [RQ, D], f32, kind="ExternalOutput").ap()

    with tile.TileContext(nc) as tc:
        # stack allocator: allocate in order of decreasing lifetime
        consts = tc.alloc_tile_pool(name="consts", bufs=1)
        o1_pool = tc.alloc_tile_pool(name="o1pool", bufs=1)
        pt_pool = tc.alloc_tile_pool(name="ptpool", bufs=1)
        p_pool = tc.alloc_tile_pool(name="ppool", bufs=6)
        s_pool = tc.alloc_tile_pool(name="spool", bufs=NMT)
        qt_pool = tc.alloc_tile_pool(name="qt", bufs=1)
        xstream = tc.alloc_tile_pool(name="xstream", bufs=4)
        wqk_pool = tc.alloc_tile_pool(name="wqkp", bufs=1)

        # constants: stats scratch, tri mask, beta
        smalls = consts.tile([128, 68], f32, name="smalls")
        beta_sb = smalls[:, 0:NKB]
        nc.sync.dma_start(beta_sb, beta_d)
        tri = consts.tile([128, NMT * 512], bf16, name="tri")
        for mt in range(NMT):
            tm = tri[:, mt * 512 : (mt + 1) * 512]
            nc.gpsimd.memset(tm, 0.0)
            # keep 0 where (mt*128 + p - y) >= 0 i.e. key y <= local row; else -1e30
            nc.gpsimd.affine_select(
                out=tm,
                in_=tm,
                compare_op=mybir.AluOpType.is_ge,
                fill=NEG,
                base=mt * 128,
                pattern=[[-1, 512]],
                channel_multiplier=1,
            )
        negmax = smalls[:, 8:12]
        lsum = smalls[:, 12:16]
        recip = smalls[:, 16:20]
        mpart = smalls[:, 20:52]
        lq = smalls[:, 52:68]

        # ---- Phase A: qT = (xq @ Wqk)^T  -> [D, RQ] in f32r --------------
        xqt_sb = wqk_pool.tile([128, KC * RQ], f32r, name="xqt_sb")
        nc.sync.dma_start(
            xqt_sb.rearrange("p (kc n) -> p kc n", kc=KC),
            xqt_d.rearrange("(kc p) n -> p kc n", p=128),
        )
        qt_sb = qt_pool.tile([128, KC * RQ], f32r, name="qt_sb")

        with (
            tc.tile_pool(name="wqkstream", bufs=3) as wqkstream,
            tc.tile_pool(name="psA", bufs=2, space="PSUM") as psA,
        ):
            for mtd in range(KC):
                wqk_blk = wqkstream.tile([128, KC * 128], f32r, name="wqk_blk", tag="wq")
                nc.sync.dma_start(
                    wqk_blk.rearrange("p (kc n) -> p kc n", kc=KC),
                    wqk_d[:, mtd * 128 : (mtd + 1) * 128].rearrange(
                        "(kc p) n -> p kc n", p=128
                    ),
                )
                ps = psA.tile([128, RQ], f32, name="ps_qt")
                for kc in range(KC):
                    nc.tensor.matmul(
                        ps[:],
                        wqk_blk[:, kc * 128 : (kc + 1) * 128],
                        xqt_sb[:, kc * RQ : (kc + 1) * RQ],
                        start=(kc == 0),
                        stop=(kc == KC - 1),
                    )
                nc.vector.tensor_copy(qt_sb[:, mtd * RQ : (mtd + 1) * RQ], ps[:])
        wqk_pool.release()

        # ---- Phase B: scores S[mt] = qT^T @ xtp + mask -------------------
        s_tiles = [s_pool.tile([128, T], f32, name=f"s_mt{mt}", tag="s") for mt in range(NMT)]
        with tc.tile_pool(name="psB", bufs=2, space="PSUM") as psB:
            for kb in range(NKB):
                halves = []
                for hh in range(2):
                    xt_h = xstream.tile(
                        [128, (KC // 2) * 512], f32r, name="xt_h", tag="xt"
                    )
                    nc.sync.dma_start(
                        xt_h.rearrange("p (kc n) -> p kc n", kc=KC // 2),
                        xtp_d[
                            hh * (D // 2) : (hh + 1) * (D // 2),
                            kb * 512 : (kb + 1) * 512,
                        ].rearrange("(kc p) n -> p kc n", p=128),
                    )
                    halves.append(xt_h)
                for mt in range(NMT):
                    ps = psB.tile([128, 512], f32, name="ps_s")
                    for kc in range(KC):
                        nc.tensor.matmul(
                            ps[:],
                            qt_sb[:, kc * RQ + mt * 128 : kc * RQ + (mt + 1) * 128],
                            halves[kc // 4][:, (kc % 4) * 512 : (kc % 4 + 1) * 512],
                            start=(kc == 0),
                            stop=(kc == KC - 1),
                        )
                    dst = s_tiles[mt][:, kb * 512 : (kb + 1) * 512]
                    if kb == 0:
                        nc.vector.tensor_add(
                            dst, ps[:], tri[:, mt * 512 : (mt + 1) * 512]
                        )
                    else:
                        nc.vector.tensor_scalar_add(
                            dst, ps[:], beta_sb[:, kb : kb + 1]
                        )
                    nc.vector.tensor_reduce(
                        mpart[:, mt * NKB + kb : mt * NKB + kb + 1],
                        dst,
                        axis=mybir.AxisListType.X,
                        op=mybir.AluOpType.max,
                    )
                    if kb == NKB - 1:
                        # finalize this row tile's (negated) max immediately
                        # so exp can start while B's remaining tiles compute
                        nc.vector.tensor_reduce(
                            negmax[:, mt : mt + 1],
                            mpart[:, mt * NKB : (mt + 1) * NKB],
                            axis=mybir.AxisListType.X,
                            op=mybir.AluOpType.max,
                            negate=True,
                        )
        xstream.release()
        qt_pool.release()

        # ---- Phase C/D: exp in quarter chunks, pipelined with DMA xbar ---
        # transposes (out[p, kc, m] = in[m, kc*128 + p]) on the ACT HWDGE
        # ring, which must carry ONLY transposes: mixing plain copies onto
        # it corrupts transfers on this stack (hw xbar-mode hazard).
        QW = T // 4  # 1024 cols per exp/transpose chunk
        pt_tiles = [
            pt_pool.tile([128, 8 * RQ], bf16, name=f"pt_q{qq}", tag=f"ptq{qq}")
            for qq in range(4)
        ]
        pt_vs = [
            ptq.rearrange("p (kc four m) -> p kc four m", kc=8, four=NMT)
            for ptq in pt_tiles
        ]
        for qq in range(4):
            for mt in range(NMT):
                p_q = p_pool.tile([128, QW], bf16, name="p_q", tag="pq")
                nc.scalar.activation(
                    p_q[:],
                    s_tiles[mt][:, qq * QW : (qq + 1) * QW],
                    mybir.ActivationFunctionType.Exp,
                    bias=negmax[:, mt : mt + 1],
                    scale=1.0,
                    accum_out=lq[:, mt * 4 + qq : mt * 4 + qq + 1],
                )
                nc.scalar.dma_start_transpose(
                    pt_vs[qq][:, :, mt, :], p_q[:]
                )
        for mt in range(NMT):
            nc.vector.tensor_reduce(
                lsum[:, mt : mt + 1],
                lq[:, mt * 4 : (mt + 1) * 4],
                axis=mybir.AxisListType.X,
                op=mybir.AluOpType.add,
            )
            nc.vector.reciprocal(recip[:, mt : mt + 1], lsum[:, mt : mt + 1])
        s_pool.release()
        p_pool.release()
        wovstream = tc.alloc_tile_pool(name="wovstream", bufs=2)

        # ---- Phase E: o1T = xp^T @ attn^T  -> [D, RQ] f32r ---------------
        o1t_sb = o1_pool.tile([128, KC * RQ], f32r, name="o1t_sb")
        with (
            tc.tile_pool(name="xpstream", bufs=3) as xpstream,
            tc.tile_pool(name="psE", bufs=2, space="PSUM") as psE,
        ):
            for mtd in range(KC):
                xpb = xpstream.tile([128, (T // 128) * 128], bf16, name="xp_blk", tag="xp")
                nc.sync.dma_start(
                    xpb.rearrange("p (kc n) -> p kc n", kc=T // 128),
                    xp_d[:, mtd * 128 : (mtd + 1) * 128].rearrange(
                        "(kc p) n -> p kc n", p=128
                    ),
                )
                ps = psE.tile([128, RQ], f32, name="ps_av")
                for kc in range(T // 128):
                    nc.tensor.matmul(
                        ps[:],
                        xpb[:, kc * 128 : (kc + 1) * 128],
                        pt_tiles[kc // 8][:, (kc % 8) * RQ : (kc % 8 + 1) * RQ],
                        start=(kc == 0),
                        stop=(kc == T // 128 - 1),
                    )
                nc.vector.tensor_copy(o1t_sb[:, mtd * RQ : (mtd + 1) * RQ], ps[:])

        # ---- Phase F: out = (o1 @ Wov) * recip ---------------------------
        with (
            tc.tile_pool(name="psF", bufs=2, space="PSUM") as psF,
            tc.tile_pool(name="outp", bufs=3) as outp,
        ):
            for nb in range(2):
                wov_blk = wovstream.tile([128, KC * 512], f32r, name="wov_blk", tag="wv")
                nc.sync.dma_start(
                    wov_blk.rearrange("p (kc n) -> p kc n", kc=KC),
                    wov_d[:, nb * 512 : (nb + 1) * 512].rearrange(
                        "(kc p) n -> p kc n", p=128
                    ),
                )
                for mt in range(NMT):
                    ps = psF.tile([128, 512], f32, name="ps_o")
                    for kc in range(KC):
                        nc.tensor.matmul(
                            ps[:],
                            o1t_sb[:, kc * RQ + mt * 128 : kc * RQ + (mt + 1) * 128],
                            wov_blk[:, kc * 512 : (kc + 1) * 512],
                            start=(kc == 0),
                            stop=(kc == KC - 1),
                        )
                    ob = outp.tile([128, 512], f32, name="ob")
                    nc.vector.tensor_scalar_mul(
                        ob[:], ps[:], recip[:, mt : mt + 1]
                    )
                    nc.sync.dma_start(
                        out_d[mt * 128 : (mt + 1) * 128, nb * 512 : (nb + 1) * 512],
                        ob[:],
                    )

        wovstream.release()
        pt_pool.release()
        o1_pool.release()
        consts.release()

    nc.compile()
    return nc


_NC_CACHE = {}


def _get_nc():
    if "nc" not in _NC_CACHE:
        _NC_CACHE["nc"] = _build_nc()
    return _NC_CACHE["nc"]


def _prep_in_maps(x, Wqk, Wov):
    x = np.ascontiguousarray(np.asarray(x), dtype=np.float32)
    Wqk = np.ascontiguousarray(np.asarray(Wqk), dtype=np.float32)
    Wov = np.ascontiguousarray(np.asarray(Wov), dtype=np.float32)
    xT = np.ascontiguousarray(x.T)
    wqk_r = _round_f32r(Wqk)
    wov_r = _round_f32r(Wov)
    xT_r = _round_f32r(xT)  # [D, T]
    x_bf = x.astype(ml_dtypes.bfloat16)

    in_maps = []
    for c in range(NCORES):
        order = [c] + [b for b in range(NKB) if b != c]
        beta_row = np.zeros(NKB, np.float32)
        for slot, b in enumerate(order):
            if b > c:
                beta_row[slot] = NEG
        xqt = _round_f32r(xT[:, c * RQ : (c + 1) * RQ])
        xtp = np.concatenate(
            [xT_r[:, b * 512 : (b + 1) * 512] for b in order], axis=1
        )
        xp = np.concatenate([x_bf[b * 512 : (b + 1) * 512, :] for b in order], axis=0)
        in_maps.append(
            {
                "xqt": np.ascontiguousarray(xqt),
                "xtp": np.ascontiguousarray(xtp),
                "xp": np.ascontiguousarray(xp),
                "wqk": wqk_r,
                "wov": wov_r,
                "beta": np.ascontiguousarray(
                    np.broadcast_to(beta_row, (128, NKB))
                ).astype(np.float32),
            }
        )
    return in_maps


def run(x, Wqk, Wov, **spmd_kwargs):
    """Full pipeline; returns (output [T, D] fp32, BassKernelResults)."""
    import time

    nc = _get_nc()
    in_maps = _prep_in_maps(x, Wqk, Wov)
    try:
        res = run_bass_kernel_spmd(
            nc, in_maps, core_ids=list(range(NCORES)), **spmd_kwargs
        )
    except Exception:
        # a prior crashed execution can leave a core transiently
        # unrecoverable; the runtime resets it — retry once
        time.sleep(10)
        res = run_bass_kernel_spmd(
            nc, in_maps, core_ids=list(range(NCORES)), **spmd_kwargs
        )
    out = np.concatenate([res.results[c]["out"] for c in range(NCORES)], axis=0)
    return np.ascontiguousarray(out, dtype=np.float32), res


def kernel(x, Wqk, Wov):
    out, _ = run(x, Wqk, Wov)
    return out



# revision 5
# speedup vs baseline: 1.0813x; 1.0813x over previous
"""Causal attention kernel for Trainium2, 8 NeuronCores, sequence-parallel.

Reference computation (T=4096, D=1024, fp32):
    q = x @ Wqk; logits = q @ x.T (causal masked); attn = softmax(logits)
    out = (attn @ x) @ Wov

Causal load balancing under one SPMD program: the 32 query row-tiles of 128
are assigned to cores as {c, 15-c, 16+c, 31-c} and host-permuted into 4
local "slots" ordered by visibility class. Slot m processes a fixed key
budget of 8*(m+1) key-tiles (keys in natural order, prefix [0, 1024*(m+1))),
which covers every core's visible range in that class. Causality inside the
budget is enforced by a host-provided additive mask (0 / -60000) that also
carries the diagonal triangle, so the program is core-independent while
skipping 37.5% of the score/AV matmul work.

Matmul precision: fp16 inputs (x, Wqk, Wov, attn) with fp32 PSUM
accumulation; q and o1 kept in fp16/f32r on-chip. Softmax row max
subtracted in fp32; attn stored fp16 for the DMA-xbar transposes and AV.
"""

import sys

sys.path.insert(0, "/opt/trn_rl_repo")

import numpy as np

import concourse.tile as tile
from concourse import bacc, mybir
from concourse.bass_utils import run_bass_kernel_spmd

T = 4096
D = 1024
NCORES = 8
RQ = T // NCORES  # 512 query rows per core
KC = D // 128  # 8 contraction chunks
NEG16 = -60000.0

BKT = [8, 16, 24, 32]  # key tiles (128) processed per slot
BG = [b // 4 for b in BKT]  # 512-wide key groups per slot
OFFK = [0, 1024, 3072, 6144]  # slot column offsets in ragged score layout
STOT = 10240  # total score/mask columns
MPOFF = [0, 2, 6, 12]  # mpart offsets (prefix of BG)
NCH = [b // 8 for b in BKT]  # 1024-wide exp chunks per slot: 1,2,3,4
LQOFF = [0, 1, 3, 6]  # lq offsets (prefix of NCH)

f32 = mybir.dt.float32
f32r = mybir.dt.float32r
f16 = mybir.dt.float16


def _build_nc():
    nc = bacc.Bacc(
        "TRN2", target_bir_lowering=False, debug=False, num_devices=NCORES
    )

    xqt_d = nc.dram_tensor("xqt", [D, RQ], f16, kind="ExternalInput").ap()
    xtp_d = nc.dram_tensor("xtp", [D, T], f16, kind="ExternalInput").ap()
    xp_d = nc.dram_tensor("xp", [T, D], f16, kind="ExternalInput").ap()
    wqk_d = nc.dram_tensor("wqk", [D, D], f16, kind="ExternalInput").ap()
    wov_d = nc.dram_tensor("wov", [D, D], f16, kind="ExternalInput").ap()
    mask_d = nc.dram_tensor("mask", [128, STOT], f16, kind="ExternalInput").ap()
    out_d = nc.dram_tensor("out", [RQ, D], f32, kind="ExternalOutput").ap()

    with tile.TileContext(nc) as tc:
        # stack allocator: long-lived pools first
        consts = tc.alloc_tile_pool(name="consts", bufs=1)
        pt_pool = tc.alloc_tile_pool(name="ptpool", bufs=1)
        o1_pool = tc.alloc_tile_pool(name="o1pool", bufs=1)
        xpstream = tc.alloc_tile_pool(name="xpstream", bufs=4)
        p_pool = tc.alloc_tile_pool(name="ppool", bufs=4)
        s_pool = tc.alloc_tile_pool(name="spool", bufs=2)
        qt_pool = tc.alloc_tile_pool(name="qt", bufs=1)
        xtp_pool = tc.alloc_tile_pool(name="xtpp", bufs=1)
        mask_pool = tc.alloc_tile_pool(name="maskp", bufs=1)

        # stats scratch: negmax 0:4, lsum 4:8, recip 8:12, mpart 12:32, lq 32:42
        smalls = consts.tile([128, 48], f32, name="smalls")
        negmax = smalls[:, 0:4]
        lsum = smalls[:, 4:8]
        recip = smalls[:, 8:12]
        mpart = smalls[:, 12:32]
        lq = smalls[:, 32:42]

        # transposed-attn tiles, ragged per slot: [keys-part, kt, 128 rows]
        pt_tiles = [
            pt_pool.tile([128, BKT[m] * 128], f16, name=f"pt{m}")
            for m in range(4)
        ]
        o1t_sb = o1_pool.tile([128, KC * RQ], f16, name="o1t_sb")

        # resident keys (natural order) and masks
        xtp_sb = xtp_pool.tile([128, KC * T], f16, name="xtp_sb")
        for kg in range(T // 512):
            nc.sync.dma_start(
                xtp_sb.rearrange("p (kc y) -> p kc y", kc=KC)[
                    :, :, kg * 512 : (kg + 1) * 512
                ],
                xtp_d[:, kg * 512 : (kg + 1) * 512].rearrange(
                    "(kc p) n -> p kc n", p=128
                ),
            )
        mask_sb = mask_pool.tile([128, STOT], f16, name="mask_sb")
        nc.sync.dma_start(mask_sb, mask_d)

        qt_sb = qt_pool.tile([128, KC * RQ], f16, name="qt_sb")

        # ---- Phase A: qT = (xq @ Wqk)^T  -> [D, RQ] fp16 ------------------
        with (
            tc.tile_pool(name="xqtp", bufs=1) as xqt_pool,
            tc.tile_pool(name="wqkstream", bufs=3) as wqkstream,
            tc.tile_pool(name="psA", bufs=2, space="PSUM") as psA,
        ):
            xqt_sb = xqt_pool.tile([128, KC * RQ], f16, name="xqt_sb")
            nc.sync.dma_start(
                xqt_sb.rearrange("p (kc n) -> p kc n", kc=KC),
                xqt_d.rearrange("(kc p) n -> p kc n", p=128),
            )
            for md2 in range(KC // 2):
                wqk_blk = wqkstream.tile(
                    [128, KC * 256], f16, name="wqk_blk", tag="wq"
                )
                nc.sync.dma_start(
                    wqk_blk.rearrange("p (kc n) -> p kc n", kc=KC),
                    wqk_d[:, md2 * 256 : (md2 + 1) * 256].rearrange(
                        "(kc p) n -> p kc n", p=128
                    ),
                )
                for h in range(2):
                    mtd = md2 * 2 + h
                    ps = psA.tile([128, RQ], f32, name="ps_qt")
                    for kc in range(KC):
                        nc.tensor.matmul(
                            ps[:],
                            wqk_blk[
                                :, kc * 256 + h * 128 : kc * 256 + h * 128 + 128
                            ],
                            xqt_sb[:, kc * RQ : (kc + 1) * RQ],
                            start=(kc == 0),
                            stop=(kc == KC - 1),
                        )
                    nc.vector.tensor_copy(
                        qt_sb[:, mtd * RQ : (mtd + 1) * RQ], ps[:]
                    )

        # ---- Phase B: per-slot scores + mask + softmax stats + exp/T -----
        with tc.tile_pool(name="psB", bufs=2, space="PSUM") as psB:
            for m in range(4):
                s_t = s_pool.tile([128, BKT[m] * 128], f32, name=f"s{m}", tag="s")
                for kg in range(BG[m]):
                    ps = psB.tile([128, 512], f32, name="ps_s", tag="psb")
                    for kc in range(KC):
                        nc.tensor.matmul(
                            ps[:],
                            qt_sb[:, kc * RQ + m * 128 : kc * RQ + (m + 1) * 128],
                            xtp_sb[:, kc * T + kg * 512 : kc * T + (kg + 1) * 512],
                            start=(kc == 0),
                            stop=(kc == KC - 1),
                        )
                    dst = s_t[:, kg * 512 : (kg + 1) * 512]
                    nc.vector.tensor_add(
                        dst,
                        ps[:],
                        mask_sb[:, OFFK[m] + kg * 512 : OFFK[m] + (kg + 1) * 512],
                    )
                    nc.vector.tensor_reduce(
                        mpart[:, MPOFF[m] + kg : MPOFF[m] + kg + 1],
                        dst,
                        axis=mybir.AxisListType.X,
                        op=mybir.AluOpType.max,
                    )
                nc.vector.tensor_reduce(
                    negmax[:, m : m + 1],
                    mpart[:, MPOFF[m] : MPOFF[m] + BG[m]],
                    axis=mybir.AxisListType.X,
                    op=mybir.AluOpType.max,
                    negate=True,
                )
                # exp in 1024-wide chunks, transposed onto the ACT xbar ring
                pt_v = pt_tiles[m].rearrange("p (kt r) -> p kt r", r=128)
                for cidx in range(NCH[m]):
                    p_q = p_pool.tile([128, 1024], f16, name="p_q", tag="pq")
                    nc.scalar.activation(
                        p_q[:],
                        s_t[:, cidx * 1024 : (cidx + 1) * 1024],
                        mybir.ActivationFunctionType.Exp,
                        bias=negmax[:, m : m + 1],
                        scale=1.0,
                        accum_out=lq[:, LQOFF[m] + cidx : LQOFF[m] + cidx + 1],
                    )
                    nc.scalar.dma_start_transpose(
                        pt_v[:, cidx * 8 : (cidx + 1) * 8, :], p_q[:]
                    )
                nc.vector.tensor_reduce(
                    lsum[:, m : m + 1],
                    lq[:, LQOFF[m] : LQOFF[m] + NCH[m]],
                    axis=mybir.AxisListType.X,
                    op=mybir.AluOpType.add,
                )
                nc.vector.reciprocal(recip[:, m : m + 1], lsum[:, m : m + 1])

        mask_pool.release()
        xtp_pool.release()
        qt_pool.release()
        s_pool.release()

        # ---- Phase E: o1T[d] = sum_kt xp[kt,d]^T @ attn^T[kt] ------------
        wovstream = tc.alloc_tile_pool(name="wovstream", bufs=2)
        with tc.tile_pool(name="psE", bufs=1, space="PSUM") as psE_pool:
            psE = [
                psE_pool.tile([128, RQ], f32, name=f"psE{d}") for d in range(KC)
            ]
            for kt in range(T // 128):
                xp_t = xpstream.tile([128, D], f16, name="xp_t", tag="xp")
                nc.sync.dma_start(xp_t[:], xp_d[kt * 128 : (kt + 1) * 128, :])
                for d in range(KC):
                    stat = xp_t[:, d * 128 : (d + 1) * 128]
                    for m in range(4):
                        if kt < BKT[m]:
                            # start_tensor_calc zeroes the WHOLE psum bank,
                            # so only the first matmul into bank d sets it;
                            # the other slot regions accumulate onto zeros.
                            nc.tensor.matmul(
                                psE[d][:, m * 128 : (m + 1) * 128],
                                stat,
                                pt_tiles[m][:, kt * 128 : (kt + 1) * 128],
                                start=(kt == 0 and m == 0),
                                stop=(kt == BKT[m] - 1),
                                skip_group_check=True,
                            )
            for d in range(KC):
                nc.vector.tensor_copy(o1t_sb[:, d * RQ : (d + 1) * RQ], psE[d][:])

        # ---- Phase F: out = (o1 @ Wov) * recip ---------------------------
        with (
            tc.tile_pool(name="psF", bufs=2, space="PSUM") as psF,
            tc.tile_pool(name="outp", bufs=3) as outp,
        ):
            for nb in range(2):
                wov_blk = wovstream.tile(
                    [128, KC * 512], f16, name="wov_blk", tag="wv"
                )
                nc.sync.dma_start(
                    wov_blk.rearrange("p (kc n) -> p kc n", kc=KC),
                    wov_d[:, nb * 512 : (nb + 1) * 512].rearrange(
                        "(kc p) n -> p kc n", p=128
                    ),
                )
                for m in range(4):
                    ps = psF.tile([128, 512], f32, name="ps_o")
                    for kc in range(KC):
                        nc.tensor.matmul(
                            ps[:],
                            o1t_sb[:, kc * RQ + m * 128 : kc * RQ + (m + 1) * 128],
                            wov_blk[:, kc * 512 : (kc + 1) * 512],
                            start=(kc == 0),
                            stop=(kc == KC - 1),
                        )
                    ob = outp.tile([128, 512], f32, name="ob")
                    nc.vector.tensor_scalar_mul(ob[:], ps[:], recip[:, m : m + 1])
                    nc.sync.dma_start(
                        out_d[m * 128 : (m + 1) * 128, nb * 512 : (nb + 1) * 512],
                        ob[:],
                    )

        wovstream.release()
        p_pool.release()
        xpstream.release()
        o1_pool.release()
        pt_pool.release()
        consts.release()

    nc.compile()
    return nc


_NC_CACHE = {}


def _get_nc():
    if "nc" not in _NC_CACHE:
        _NC_CACHE["nc"] = _build_nc()
    return _NC_CACHE["nc"]


def _slot_tiles(c):
    return [c, 15 - c, 16 + c, 31 - c]


def _prep_in_maps(x, Wqk, Wov):
    x = np.ascontiguousarray(np.asarray(x), dtype=np.float32)
    Wqk = np.ascontiguousarray(np.asarray(Wqk), dtype=np.float32)
    Wov = np.ascontiguousarray(np.asarray(Wov), dtype=np.float32)
    x16 = x.astype(np.float16)
    xT16 = np.ascontiguousarray(x16.T)  # [D, T]
    wqk16 = Wqk.astype(np.float16)
    wov16 = Wov.astype(np.float16)

    in_maps = []
    for c in range(NCORES):
        tiles = _slot_tiles(c)
        rows = np.concatenate(
            [np.arange(t * 128, (t + 1) * 128) for t in tiles]
        )
        xqt = np.ascontiguousarray(xT16[:, rows])
        mask = np.full((128, STOT), NEG16, dtype=np.float16)
        p = np.arange(128)[:, None]
        for m, t in enumerate(tiles):
            g = t * 128 + p  # global row index per partition
            y = np.arange(BKT[m] * 128)[None, :]  # global key index
            mask[:, OFFK[m] : OFFK[m] + BKT[m] * 128] = np.where(
                y <= g, np.float16(0.0), np.float16(NEG16)
            )
        in_maps.append(
            {
                "xqt": xqt,
                "xtp": xT16,
                "xp": x16,
                "wqk": wqk16,
                "wov": wov16,
                "mask": mask,
            }
        )
    return in_maps


def run(x, Wqk, Wov, **spmd_kwargs):
    """Full pipeline; returns (output [T, D] fp32, BassKernelResults)."""
    import time

    nc = _get_nc()
    in_maps = _prep_in_maps(x, Wqk, Wov)
    try:
        res = run_bass_kernel_spmd(
            nc, in_maps, core_ids=list(range(NCORES)), **spmd_kwargs
        )
    except Exception:
        # a prior crashed execution can leave a core transiently
        # unrecoverable; the runtime resets it — retry once
        time.sleep(10)
        res = run_bass_kernel_spmd(
            nc, in_maps, core_ids=list(range(NCORES)), **spmd_kwargs
        )
    out = np.empty((T, D), dtype=np.float32)
    for c in range(NCORES):
        co = res.results[c]["out"]
        for m, t in enumerate(_slot_tiles(c)):
            out[t * 128 : (t + 1) * 128] = co[m * 128 : (m + 1) * 128]
    return np.ascontiguousarray(out), res


def kernel(x, Wqk, Wov):
    out, _ = run(x, Wqk, Wov)
    return out


# revision 6
# speedup vs baseline: 1.5243x; 1.4096x over previous
"""Causal attention kernel for Trainium2, 8 NeuronCores, sequence-parallel.

Reference computation (T=4096, D=1024, fp32):
    q = x @ Wqk; logits = q @ x.T (causal masked); attn = softmax(logits)
    out = (attn @ x) @ Wov

Causal load balancing under one SPMD program: the 32 query row-tiles of 128
are assigned to cores as {c, 15-c, 16+c, 31-c} and host-permuted into 4
local "slots" ordered by visibility class. Slot m processes a fixed key
budget of 8*(m+1) key-tiles (keys in natural order, prefix [0, 1024*(m+1))),
which covers every core's visible range in that class. Causality inside the
budget is enforced by a host-provided additive mask (0 / -60000) that also
carries the diagonal triangle, so the program is core-independent while
skipping 37.5% of the score/AV matmul work.

Matmul precision: fp16 inputs (x, Wqk, Wov, attn) with fp32 PSUM
accumulation; q and o1 kept in fp16 on-chip. Softmax row max subtracted in
fp32; attn stored fp16 for the DMA-xbar transposes and AV.

Scheduling notes: input DMAs are issued in consumption order (xqt/wqk for
phase A first, then keys/masks); phase B runs slots largest-first so the
exp/transpose pipeline drains during B and phase E can start right after;
tiles are split per dependency unit (per-kg keys, per-chunk attn-transpose,
per-d o1) to keep cross-engine waits granular.
"""

import sys

sys.path.insert(0, "/opt/trn_rl_repo")

import numpy as np

import concourse.tile as tile
from concourse import bacc, mybir
from concourse.bass_utils import run_bass_kernel_spmd

T = 4096
D = 1024
NCORES = 8
RQ = T // NCORES  # 512 query rows per core
KC = D // 128  # 8 contraction chunks
NEG16 = -60000.0

BKT = [8, 16, 24, 32]  # key tiles (128) processed per slot
BG = [b // 4 for b in BKT]  # 512-wide key groups per slot
OFFK = [0, 1024, 3072, 6144]  # slot column offsets in ragged score layout
STOT = 10240  # total score/mask columns
MPOFF = [0, 2, 6, 12]  # mpart offsets (prefix of BG)
NCH = [b // 8 for b in BKT]  # 1024-wide exp chunks per slot: 1,2,3,4
LQOFF = [0, 1, 3, 6]  # lq offsets (prefix of NCH)

f32 = mybir.dt.float32
f16 = mybir.dt.float16


def _build_nc():
    nc = bacc.Bacc(
        "TRN2", target_bir_lowering=False, debug=False, num_devices=NCORES
    )

    xqt_d = nc.dram_tensor("xqt", [D, RQ], f16, kind="ExternalInput").ap()
    xtp_d = nc.dram_tensor("xtp", [D, T], f16, kind="ExternalInput").ap()
    xp_d = nc.dram_tensor("xp", [T, D], f16, kind="ExternalInput").ap()
    wqk_d = nc.dram_tensor("wqk", [D, D], f16, kind="ExternalInput").ap()
    wov_d = nc.dram_tensor("wov", [D, D], f16, kind="ExternalInput").ap()
    mask_d = nc.dram_tensor("mask", [128, STOT], f16, kind="ExternalInput").ap()
    out_d = nc.dram_tensor("out", [RQ, D], f32, kind="ExternalOutput").ap()

    with tile.TileContext(nc) as tc:
        # stack allocator: long-lived pools first
        consts = tc.alloc_tile_pool(name="consts", bufs=1)
        pt_pool = tc.alloc_tile_pool(name="ptpool", bufs=1)
        o1_pool = tc.alloc_tile_pool(name="o1pool", bufs=1)
        xpstream = tc.alloc_tile_pool(name="xpstream", bufs=8)
        p_pool = tc.alloc_tile_pool(name="ppool", bufs=4)
        s_pool = tc.alloc_tile_pool(name="spool", bufs=2)
        qt_pool = tc.alloc_tile_pool(name="qt", bufs=1)
        xtp_pool = tc.alloc_tile_pool(name="xtpp", bufs=1)
        mask_pool = tc.alloc_tile_pool(name="maskp", bufs=1)
        wqk_pool = tc.alloc_tile_pool(name="wqkp", bufs=1)
        xqt_pool = tc.alloc_tile_pool(name="xqtp", bufs=1)

        # stats scratch: negmax 0:4, lsum 4:8, recip 8:12, mpart 12:32, lq 32:42
        smalls = consts.tile([128, 48], f32, name="smalls")
        negmax = smalls[:, 0:4]
        lsum = smalls[:, 4:8]
        recip = smalls[:, 8:12]
        mpart = smalls[:, 12:32]
        lq = smalls[:, 32:42]

        # transposed-attn chunk tiles [keys-part, kt_local(8), 128 rows]
        ptc = [
            [
                pt_pool.tile([128, 1024], f16, name=f"pt{m}_{c}")
                for c in range(NCH[m])
            ]
            for m in range(4)
        ]
        o1t = [o1_pool.tile([128, RQ], f16, name=f"o1t{d}") for d in range(KC)]
        qt_sb = qt_pool.tile([128, KC * RQ], f16, name="qt_sb")
        xtp_t = [
            xtp_pool.tile([128, KC * 512], f16, name=f"xtp{kg}")
            for kg in range(T // 512)
        ]
        mask_t = [
            mask_pool.tile([128, BKT[m] * 128], f16, name=f"mask{m}")
            for m in range(4)
        ]
        wqk_t = [
            wqk_pool.tile([128, KC * 256], f16, name=f"wqk{md2}")
            for md2 in range(KC // 2)
        ]
        xqt_sb = xqt_pool.tile([128, KC * RQ], f16, name="xqt_sb")

        # ---- input DMAs, issued in consumption order ---------------------
        def load_wqk(md2):
            nc.sync.dma_start(
                wqk_t[md2].rearrange("p (kc n) -> p kc n", kc=KC),
                wqk_d[:, md2 * 256 : (md2 + 1) * 256].rearrange(
                    "(kc p) n -> p kc n", p=128
                ),
            )

        def load_xtp(kg):
            nc.sync.dma_start(
                xtp_t[kg].rearrange("p (kc n) -> p kc n", kc=KC),
                xtp_d[:, kg * 512 : (kg + 1) * 512].rearrange(
                    "(kc p) n -> p kc n", p=128
                ),
            )

        def load_mask(m):
            nc.sync.dma_start(
                mask_t[m], mask_d[:, OFFK[m] : OFFK[m] + BKT[m] * 128]
            )

        nc.sync.dma_start(
            xqt_sb.rearrange("p (kc n) -> p kc n", kc=KC),
            xqt_d.rearrange("(kc p) n -> p kc n", p=128),
        )
        load_wqk(0)
        load_wqk(1)
        load_mask(3)
        load_wqk(2)
        load_xtp(0)
        load_wqk(3)
        load_xtp(1)
        for kg in range(2, 8):
            load_xtp(kg)
        load_mask(2)
        load_mask(1)
        load_mask(0)

        # ---- Phase A: qT = (xq @ Wqk)^T  -> [D, RQ] fp16 -----------------
        with tc.tile_pool(name="psA", bufs=2, space="PSUM") as psA:
            for md2 in range(KC // 2):
                for h in range(2):
                    mtd = md2 * 2 + h
                    ps = psA.tile([128, RQ], f32, name="ps_qt")
                    for kc in range(KC):
                        nc.tensor.matmul(
                            ps[:],
                            wqk_t[md2][
                                :, kc * 256 + h * 128 : kc * 256 + h * 128 + 128
                            ],
                            xqt_sb[:, kc * RQ : (kc + 1) * RQ],
                            start=(kc == 0),
                            stop=(kc == KC - 1),
                        )
                    nc.vector.tensor_copy(
                        qt_sb[:, mtd * RQ : (mtd + 1) * RQ], ps[:]
                    )

        # ---- Phase B: per-slot scores + mask + softmax stats + exp/T -----
        # slots largest-first so the exp/transpose pipeline overlaps B
        with tc.tile_pool(name="psB", bufs=4, space="PSUM") as psB:
            for m in (3, 2, 1, 0):
                s_t = s_pool.tile([128, BKT[m] * 128], f32, name=f"s{m}", tag="s")
                for kg in range(BG[m]):
                    ps = psB.tile([128, 512], f32, name="ps_s", tag="psb")
                    for kc in range(KC):
                        nc.tensor.matmul(
                            ps[:],
                            qt_sb[:, kc * RQ + m * 128 : kc * RQ + (m + 1) * 128],
                            xtp_t[kg][:, kc * 512 : (kc + 1) * 512],
                            start=(kc == 0),
                            stop=(kc == KC - 1),
                        )
                    dst = s_t[:, kg * 512 : (kg + 1) * 512]
                    nc.vector.tensor_add(
                        dst, ps[:], mask_t[m][:, kg * 512 : (kg + 1) * 512]
                    )
                    nc.vector.tensor_reduce(
                        mpart[:, MPOFF[m] + kg : MPOFF[m] + kg + 1],
                        dst,
                        axis=mybir.AxisListType.X,
                        op=mybir.AluOpType.max,
                    )
                nc.vector.tensor_reduce(
                    negmax[:, m : m + 1],
                    mpart[:, MPOFF[m] : MPOFF[m] + BG[m]],
                    axis=mybir.AxisListType.X,
                    op=mybir.AluOpType.max,
                    negate=True,
                )
                # exp in 1024-wide chunks, transposed onto the ACT xbar ring
                for cidx in range(NCH[m]):
                    p_q = p_pool.tile([128, 1024], f16, name="p_q", tag="pq")
                    nc.scalar.activation(
                        p_q[:],
                        s_t[:, cidx * 1024 : (cidx + 1) * 1024],
                        mybir.ActivationFunctionType.Exp,
                        bias=negmax[:, m : m + 1],
                        scale=1.0,
                        accum_out=lq[:, LQOFF[m] + cidx : LQOFF[m] + cidx + 1],
                    )
                    nc.scalar.dma_start_transpose(
                        ptc[m][cidx].rearrange("p (kt r) -> p kt r", r=128),
                        p_q[:],
                    )
                nc.vector.tensor_reduce(
                    lsum[:, m : m + 1],
                    lq[:, LQOFF[m] : LQOFF[m] + NCH[m]],
                    axis=mybir.AxisListType.X,
                    op=mybir.AluOpType.add,
                )
                nc.vector.reciprocal(recip[:, m : m + 1], lsum[:, m : m + 1])

        xqt_pool.release()
        wqk_pool.release()
        mask_pool.release()
        xtp_pool.release()
        qt_pool.release()
        s_pool.release()

        # ---- Phase E: o1T[d] = sum_kt xp[kt,d]^T @ attn^T[kt] ------------
        wovstream = tc.alloc_tile_pool(name="wovstream", bufs=2)
        with tc.tile_pool(name="psE", bufs=1, space="PSUM") as psE_pool:
            psE = [
                psE_pool.tile([128, RQ], f32, name=f"psE{d}") for d in range(KC)
            ]
            for kt in range(T // 128):
                xp_t = xpstream.tile([128, D], f16, name="xp_t", tag="xp")
                nc.sync.dma_start(xp_t[:], xp_d[kt * 128 : (kt + 1) * 128, :])
                for d in range(KC):
                    stat = xp_t[:, d * 128 : (d + 1) * 128]
                    for m in range(4):
                        if kt < BKT[m]:
                            # start_tensor_calc zeroes the WHOLE psum bank,
                            # so only the first matmul into bank d sets it;
                            # the other slot regions accumulate onto zeros.
                            nc.tensor.matmul(
                                psE[d][:, m * 128 : (m + 1) * 128],
                                stat,
                                ptc[m][kt // 8][
                                    :, (kt % 8) * 128 : (kt % 8 + 1) * 128
                                ],
                                start=(kt == 0 and m == 0),
                                stop=(kt == BKT[m] - 1),
                                skip_group_check=True,
                            )
            # evacuate: split across DVE and Act so phase F starts sooner
            for d in range(KC):
                if d % 2 == 0:
                    nc.vector.tensor_copy(o1t[d][:], psE[d][:])
                else:
                    nc.scalar.activation(
                        o1t[d][:],
                        psE[d][:],
                        mybir.ActivationFunctionType.Copy,
                    )

        # ---- Phase F: out = (o1 @ Wov) * recip ---------------------------
        with (
            tc.tile_pool(name="psF", bufs=2, space="PSUM") as psF,
            tc.tile_pool(name="outp", bufs=3) as outp,
        ):
            for nb in range(2):
                wov_blk = wovstream.tile(
                    [128, KC * 512], f16, name="wov_blk", tag="wv"
                )
                nc.sync.dma_start(
                    wov_blk.rearrange("p (kc n) -> p kc n", kc=KC),
                    wov_d[:, nb * 512 : (nb + 1) * 512].rearrange(
                        "(kc p) n -> p kc n", p=128
                    ),
                )
                for m in range(4):
                    ps = psF.tile([128, 512], f32, name="ps_o")
                    for kc in range(KC):
                        nc.tensor.matmul(
                            ps[:],
                            o1t[kc][:, m * 128 : (m + 1) * 128],
                            wov_blk[:, kc * 512 : (kc + 1) * 512],
                            start=(kc == 0),
                            stop=(kc == KC - 1),
                        )
                    ob = outp.tile([128, 512], f32, name="ob")
                    nc.vector.tensor_scalar_mul(ob[:], ps[:], recip[:, m : m + 1])
                    nc.sync.dma_start(
                        out_d[m * 128 : (m + 1) * 128, nb * 512 : (nb + 1) * 512],
                        ob[:],
                    )

        wovstream.release()
        p_pool.release()
        xpstream.release()
        o1_pool.release()
        pt_pool.release()
        consts.release()

    nc.compile()
    return nc


_NC_CACHE = {}


def _get_nc():
    if "nc" not in _NC_CACHE:
        _NC_CACHE["nc"] = _build_nc()
    return _NC_CACHE["nc"]


def _slot_tiles(c):
    return [c, 15 - c, 16 + c, 31 - c]


def _prep_in_maps(x, Wqk, Wov):
    x = np.ascontiguousarray(np.asarray(x), dtype=np.float32)
    Wqk = np.ascontiguousarray(np.asarray(Wqk), dtype=np.float32)
    Wov = np.ascontiguousarray(np.asarray(Wov), dtype=np.float32)
    x16 = x.astype(np.float16)
    xT16 = np.ascontiguousarray(x16.T)  # [D, T]
    wqk16 = Wqk.astype(np.float16)
    wov16 = Wov.astype(np.float16)

    in_maps = []
    for c in range(NCORES):
        tiles = _slot_tiles(c)
        rows = np.concatenate(
            [np.arange(t * 128, (t + 1) * 128) for t in tiles]
        )
        xqt = np.ascontiguousarray(xT16[:, rows])
        mask = np.full((128, STOT), NEG16, dtype=np.float16)
        p = np.arange(128)[:, None]
        for m, t in enumerate(tiles):
            g = t * 128 + p  # global row index per partition
            y = np.arange(BKT[m] * 128)[None, :]  # global key index
            mask[:, OFFK[m] : OFFK[m] + BKT[m] * 128] = np.where(
                y <= g, np.float16(0.0), np.float16(NEG16)
            )
        in_maps.append(
            {
                "xqt": xqt,
                "xtp": xT16,
                "xp": x16,
                "wqk": wqk16,
                "wov": wov16,
                "mask": mask,
            }
        )
    return in_maps


def run(x, Wqk, Wov, **spmd_kwargs):
    """Full pipeline; returns (output [T, D] fp32, BassKernelResults)."""
    import time

    nc = _get_nc()
    in_maps = _prep_in_maps(x, Wqk, Wov)
    try:
        res = run_bass_kernel_spmd(
            nc, in_maps, core_ids=list(range(NCORES)), **spmd_kwargs
        )
    except Exception:
        # a prior crashed execution can leave a core transiently
        # unrecoverable; the runtime resets it — retry once
        time.sleep(10)
        res = run_bass_kernel_spmd(
            nc, in_maps, core_ids=list(range(NCORES)), **spmd_kwargs
        )
    out = np.empty((T, D), dtype=np.float32)
    for c in range(NCORES):
        co = res.results[c]["out"]
        for m, t in enumerate(_slot_tiles(c)):
            out[t * 128 : (t + 1) * 128] = co[m * 128 : (m + 1) * 128]
    return np.ascontiguousarray(out), res


def kernel(x, Wqk, Wov):
    out, _ = run(x, Wqk, Wov)
    return out


# revision 7
# speedup vs baseline: 1.5562x; 1.0209x over previous
"""Causal attention kernel for Trainium2, 8 NeuronCores, sequence-parallel.

Reference computation (T=4096, D=1024, fp32):
    q = x @ Wqk; logits = q @ x.T (causal masked); attn = softmax(logits)
    out = (attn @ x) @ Wov

Causal load balancing under one SPMD program: the 32 query row-tiles of 128
are assigned to cores as {c, 15-c, 16+c, 31-c} and host-permuted into 4
local "slots" ordered by visibility class. Slot m processes a fixed key
budget of 8*(m+1) key-tiles (keys in natural order, prefix [0, 1024*(m+1))),
which covers every core's visible range in that class. Causality inside the
budget is enforced by a host-provided additive mask (0 / -60000) that also
carries the diagonal triangle, so the program is core-independent while
skipping 37.5% of the score/AV matmul work.

Matmul precision: fp16 inputs (x, Wqk, Wov, attn) with fp32 PSUM
accumulation; q and o1 kept in fp16 on-chip. Softmax row max subtracted in
fp32; attn stored fp16 for the DMA-xbar transposes and AV.

Scheduling notes: input DMAs are issued in consumption order (xqt/wqk for
phase A first, then keys/masks); phase B runs slots largest-first so the
exp/transpose pipeline drains during B and phase E can start right after;
tiles are split per dependency unit (per-kg keys, per-chunk attn-transpose,
per-d o1) to keep cross-engine waits granular.
"""

import sys

sys.path.insert(0, "/opt/trn_rl_repo")

import numpy as np

import concourse.tile as tile
from concourse import bacc, mybir
from concourse.bass_utils import run_bass_kernel_spmd

T = 4096
D = 1024
NCORES = 8
RQ = T // NCORES  # 512 query rows per core
KC = D // 128  # 8 contraction chunks
NEG16 = -60000.0

BKT = [8, 16, 24, 32]  # key tiles (128) processed per slot
BG = [b // 4 for b in BKT]  # 512-wide key groups per slot
OFFK = [0, 1024, 3072, 6144]  # slot column offsets in ragged score layout
STOT = 10240  # total score/mask columns
MPOFF = [0, 2, 6, 12]  # mpart offsets (prefix of BG)
NCH = [b // 8 for b in BKT]  # 1024-wide exp chunks per slot: 1,2,3,4
LQOFF = [0, 1, 3, 6]  # lq offsets (prefix of NCH)

f32 = mybir.dt.float32
f16 = mybir.dt.float16


def _build_nc():
    nc = bacc.Bacc(
        "TRN2", target_bir_lowering=False, debug=False, num_devices=NCORES
    )

    xqt_d = nc.dram_tensor("xqt", [D, RQ], f16, kind="ExternalInput").ap()
    xtp_d = nc.dram_tensor("xtp", [D, T], f16, kind="ExternalInput").ap()
    xp_d = nc.dram_tensor("xp", [T, D], f16, kind="ExternalInput").ap()
    wqk_d = nc.dram_tensor("wqk", [D, D], f16, kind="ExternalInput").ap()
    wov_d = nc.dram_tensor("wov", [D, D], f16, kind="ExternalInput").ap()
    mask_d = nc.dram_tensor("mask", [128, STOT], f16, kind="ExternalInput").ap()
    out_d = nc.dram_tensor("out", [RQ, D], f32, kind="ExternalOutput").ap()

    with tile.TileContext(nc) as tc:
        # stack allocator: long-lived pools first
        consts = tc.alloc_tile_pool(name="consts", bufs=1)
        pt_pool = tc.alloc_tile_pool(name="ptpool", bufs=1)
        o1_pool = tc.alloc_tile_pool(name="o1pool", bufs=1)
        xpstream = tc.alloc_tile_pool(name="xpstream", bufs=8)
        p_pool = tc.alloc_tile_pool(name="ppool", bufs=4)
        s_pool = tc.alloc_tile_pool(name="spool", bufs=2)
        qt_pool = tc.alloc_tile_pool(name="qt", bufs=1)
        xtp_pool = tc.alloc_tile_pool(name="xtpp", bufs=1)
        mask_pool = tc.alloc_tile_pool(name="maskp", bufs=1)
        wqk_pool = tc.alloc_tile_pool(name="wqkp", bufs=1)
        xqt_pool = tc.alloc_tile_pool(name="xqtp", bufs=1)

        # stats scratch: negmax 0:4, lsum 4:8, recip 8:12, mpart 12:32, lq 32:42
        smalls = consts.tile([128, 48], f32, name="smalls")
        negmax = smalls[:, 0:4]
        lsum = smalls[:, 4:8]
        recip = smalls[:, 8:12]
        mpart = smalls[:, 12:32]
        lq = smalls[:, 32:42]

        # transposed-attn chunk tiles [keys-part, kt_local(8), 128 rows]
        ptc = [
            [
                pt_pool.tile([128, 1024], f16, name=f"pt{m}_{c}")
                for c in range(NCH[m])
            ]
            for m in range(4)
        ]
        o1t = [o1_pool.tile([128, RQ], f16, name=f"o1t{d}") for d in range(KC)]
        qt_sb = qt_pool.tile([128, KC * RQ], f16, name="qt_sb")
        xtp_t = [
            xtp_pool.tile([128, KC * 512], f16, name=f"xtp{kg}")
            for kg in range(T // 512)
        ]
        mask_t = [
            mask_pool.tile([128, BKT[m] * 128], f16, name=f"mask{m}")
            for m in range(4)
        ]
        wqk_t = [
            wqk_pool.tile([128, KC * 256], f16, name=f"wqk{md2}")
            for md2 in range(KC // 2)
        ]
        xqt_sb = xqt_pool.tile([128, KC * RQ], f16, name="xqt_sb")

        # ---- input DMAs, issued in consumption order ---------------------
        def load_wqk(md2):
            nc.sync.dma_start(
                wqk_t[md2].rearrange("p (kc n) -> p kc n", kc=KC),
                wqk_d[:, md2 * 256 : (md2 + 1) * 256].rearrange(
                    "(kc p) n -> p kc n", p=128
                ),
            )

        def load_xtp(kg):
            nc.sync.dma_start(
                xtp_t[kg].rearrange("p (kc n) -> p kc n", kc=KC),
                xtp_d[:, kg * 512 : (kg + 1) * 512].rearrange(
                    "(kc p) n -> p kc n", p=128
                ),
            )

        def load_mask(m):
            nc.sync.dma_start(
                mask_t[m], mask_d[:, OFFK[m] : OFFK[m] + BKT[m] * 128]
            )

        nc.sync.dma_start(
            xqt_sb.rearrange("p (kc n) -> p kc n", kc=KC),
            xqt_d.rearrange("(kc p) n -> p kc n", p=128),
        )
        load_wqk(0)
        load_wqk(1)
        load_mask(3)
        load_wqk(2)
        load_xtp(0)
        load_wqk(3)
        load_xtp(1)
        for kg in range(2, 8):
            load_xtp(kg)
        load_mask(2)
        load_mask(1)
        load_mask(0)

        # ---- Phase A: qT = (xq @ Wqk)^T  -> [D, RQ] fp16 -----------------
        with tc.tile_pool(name="psA", bufs=2, space="PSUM") as psA:
            for md2 in range(KC // 2):
                for h in range(2):
                    mtd = md2 * 2 + h
                    ps = psA.tile([128, RQ], f32, name="ps_qt")
                    for kc in range(KC):
                        nc.tensor.matmul(
                            ps[:],
                            wqk_t[md2][
                                :, kc * 256 + h * 128 : kc * 256 + h * 128 + 128
                            ],
                            xqt_sb[:, kc * RQ : (kc + 1) * RQ],
                            start=(kc == 0),
                            stop=(kc == KC - 1),
                        )
                    nc.vector.tensor_copy(
                        qt_sb[:, mtd * RQ : (mtd + 1) * RQ], ps[:]
                    )

        # ---- Phase B: per-slot scores + mask + softmax stats + exp/T -----
        # slots largest-first so the exp/transpose pipeline overlaps B
        with tc.tile_pool(name="psB", bufs=4, space="PSUM") as psB:
            for m in (3, 2, 1, 0):
                s_t = s_pool.tile([128, BKT[m] * 128], f32, name=f"s{m}", tag="s")
                for kg in range(BG[m]):
                    ps = psB.tile([128, 512], f32, name="ps_s", tag="psb")
                    for kc in range(KC):
                        nc.tensor.matmul(
                            ps[:],
                            qt_sb[:, kc * RQ + m * 128 : kc * RQ + (m + 1) * 128],
                            xtp_t[kg][:, kc * 512 : (kc + 1) * 512],
                            start=(kc == 0),
                            stop=(kc == KC - 1),
                        )
                    dst = s_t[:, kg * 512 : (kg + 1) * 512]
                    nc.vector.tensor_add(
                        dst, ps[:], mask_t[m][:, kg * 512 : (kg + 1) * 512]
                    )
                    nc.vector.tensor_reduce(
                        mpart[:, MPOFF[m] + kg : MPOFF[m] + kg + 1],
                        dst,
                        axis=mybir.AxisListType.X,
                        op=mybir.AluOpType.max,
                    )
                nc.vector.tensor_reduce(
                    negmax[:, m : m + 1],
                    mpart[:, MPOFF[m] : MPOFF[m] + BG[m]],
                    axis=mybir.AxisListType.X,
                    op=mybir.AluOpType.max,
                    negate=True,
                )
                # exp in 1024-wide chunks, transposed onto the ACT xbar ring
                for cidx in range(NCH[m]):
                    p_q = p_pool.tile([128, 1024], f16, name="p_q", tag="pq")
                    nc.scalar.activation(
                        p_q[:],
                        s_t[:, cidx * 1024 : (cidx + 1) * 1024],
                        mybir.ActivationFunctionType.Exp,
                        bias=negmax[:, m : m + 1],
                        scale=1.0,
                        accum_out=lq[:, LQOFF[m] + cidx : LQOFF[m] + cidx + 1],
                    )
                    nc.scalar.dma_start_transpose(
                        ptc[m][cidx].rearrange("p (kt r) -> p kt r", r=128),
                        p_q[:],
                    )
                nc.vector.tensor_reduce(
                    lsum[:, m : m + 1],
                    lq[:, LQOFF[m] : LQOFF[m] + NCH[m]],
                    axis=mybir.AxisListType.X,
                    op=mybir.AluOpType.add,
                )
                nc.vector.reciprocal(recip[:, m : m + 1], lsum[:, m : m + 1])

        xqt_pool.release()
        wqk_pool.release()
        mask_pool.release()
        xtp_pool.release()
        qt_pool.release()
        s_pool.release()

        # ---- Phase E: o1T[d] = sum_kt xp[kt,d]^T @ attn^T[kt] ------------
        wovstream = tc.alloc_tile_pool(name="wovstream", bufs=2)
        with tc.tile_pool(name="psE", bufs=1, space="PSUM") as psE_pool:
            psE = [
                psE_pool.tile([128, RQ], f32, name=f"psE{d}") for d in range(KC)
            ]
            # kts 8..31 first (need only slots 1-3, whose exps finish during
            # B since B runs slots largest-first); kts 0..7 last, by which
            # time slot0's post-B exp/transpose has landed. Removes the B->E
            # pipeline bubble.
            kt_order = list(range(8, T // 128)) + list(range(8))
            for kti, kt in enumerate(kt_order):
                xp_t = xpstream.tile([128, D], f16, name="xp_t", tag="xp")
                nc.sync.dma_start(xp_t[:], xp_d[kt * 128 : (kt + 1) * 128, :])
                for d in range(KC):
                    stat = xp_t[:, d * 128 : (d + 1) * 128]
                    for m in range(4):
                        if kt < BKT[m]:
                            # start_tensor_calc zeroes the WHOLE psum bank,
                            # so only the first matmul into bank d sets it;
                            # the other slot regions accumulate onto zeros.
                            # All chains end in the final kt block (0..7).
                            nc.tensor.matmul(
                                psE[d][:, m * 128 : (m + 1) * 128],
                                stat,
                                ptc[m][kt // 8][
                                    :, (kt % 8) * 128 : (kt % 8 + 1) * 128
                                ],
                                start=(kti == 0 and m == 1),
                                stop=(kt == 7),
                                skip_group_check=True,
                            )
            # evacuate: split across DVE and Act so phase F starts sooner
            for d in range(KC):
                if d % 2 == 0:
                    nc.vector.tensor_copy(o1t[d][:], psE[d][:])
                else:
                    nc.scalar.activation(
                        o1t[d][:],
                        psE[d][:],
                        mybir.ActivationFunctionType.Copy,
                    )

        # ---- Phase F: out = (o1 @ Wov) * recip ---------------------------
        with (
            tc.tile_pool(name="psF", bufs=2, space="PSUM") as psF,
            tc.tile_pool(name="outp", bufs=3) as outp,
        ):
            for nb in range(2):
                wov_blk = wovstream.tile(
                    [128, KC * 512], f16, name="wov_blk", tag="wv"
                )
                nc.sync.dma_start(
                    wov_blk.rearrange("p (kc n) -> p kc n", kc=KC),
                    wov_d[:, nb * 512 : (nb + 1) * 512].rearrange(
                        "(kc p) n -> p kc n", p=128
                    ),
                )
                for m in range(4):
                    ps = psF.tile([128, 512], f32, name="ps_o")
                    for kc in range(KC):
                        nc.tensor.matmul(
                            ps[:],
                            o1t[kc][:, m * 128 : (m + 1) * 128],
                            wov_blk[:, kc * 512 : (kc + 1) * 512],
                            start=(kc == 0),
                            stop=(kc == KC - 1),
                        )
                    ob = outp.tile([128, 512], f32, name="ob")
                    nc.vector.tensor_scalar_mul(ob[:], ps[:], recip[:, m : m + 1])
                    nc.sync.dma_start(
                        out_d[m * 128 : (m + 1) * 128, nb * 512 : (nb + 1) * 512],
                        ob[:],
                    )

        wovstream.release()
        p_pool.release()
        xpstream.release()
        o1_pool.release()
        pt_pool.release()
        consts.release()

    nc.compile()
    return nc


_NC_CACHE = {}


def _get_nc():
    if "nc" not in _NC_CACHE:
        _NC_CACHE["nc"] = _build_nc()
    return _NC_CACHE["nc"]


def _slot_tiles(c):
    return [c, 15 - c, 16 + c, 31 - c]


def _prep_in_maps(x, Wqk, Wov):
    x = np.ascontiguousarray(np.asarray(x), dtype=np.float32)
    Wqk = np.ascontiguousarray(np.asarray(Wqk), dtype=np.float32)
    Wov = np.ascontiguousarray(np.asarray(Wov), dtype=np.float32)
    x16 = x.astype(np.float16)
    xT16 = np.ascontiguousarray(x16.T)  # [D, T]
    wqk16 = Wqk.astype(np.float16)
    wov16 = Wov.astype(np.float16)

    in_maps = []
    for c in range(NCORES):
        tiles = _slot_tiles(c)
        rows = np.concatenate(
            [np.arange(t * 128, (t + 1) * 128) for t in tiles]
        )
        xqt = np.ascontiguousarray(xT16[:, rows])
        mask = np.full((128, STOT), NEG16, dtype=np.float16)
        p = np.arange(128)[:, None]
        for m, t in enumerate(tiles):
            g = t * 128 + p  # global row index per partition
            y = np.arange(BKT[m] * 128)[None, :]  # global key index
            mask[:, OFFK[m] : OFFK[m] + BKT[m] * 128] = np.where(
                y <= g, np.float16(0.0), np.float16(NEG16)
            )
        in_maps.append(
            {
                "xqt": xqt,
                "xtp": xT16,
                "xp": x16,
                "wqk": wqk16,
                "wov": wov16,
                "mask": mask,
            }
        )
    return in_maps


def run(x, Wqk, Wov, **spmd_kwargs):
    """Full pipeline; returns (output [T, D] fp32, BassKernelResults)."""
    import time

    nc = _get_nc()
    in_maps = _prep_in_maps(x, Wqk, Wov)
    try:
        res = run_bass_kernel_spmd(
            nc, in_maps, core_ids=list(range(NCORES)), **spmd_kwargs
        )
    except Exception:
        # a prior crashed execution can leave a core transiently
        # unrecoverable; the runtime resets it — retry once
        time.sleep(10)
        res = run_bass_kernel_spmd(
            nc, in_maps, core_ids=list(range(NCORES)), **spmd_kwargs
        )
    out = np.empty((T, D), dtype=np.float32)
    for c in range(NCORES):
        co = res.results[c]["out"]
        for m, t in enumerate(_slot_tiles(c)):
            out[t * 128 : (t + 1) * 128] = co[m * 128 : (m + 1) * 128]
    return np.ascontiguousarray(out), res


def kernel(x, Wqk, Wov):
    out, _ = run(x, Wqk, Wov)
    return out


# revision 11
# speedup vs baseline: 1.5660x; 1.0063x over previous
"""Causal attention kernel for Trainium2, 8 NeuronCores, sequence-parallel.

Reference computation (T=4096, D=1024, fp32):
    q = x @ Wqk; logits = q @ x.T (causal masked); attn = softmax(logits)
    out = (attn @ x) @ Wov

Causal load balancing under one SPMD program: the 32 query row-tiles of 128
are assigned to cores as {c, 15-c, 16+c, 31-c} and host-permuted into 4
local "slots" ordered by visibility class. Slot m processes a fixed key
budget of 8*(m+1) key-tiles (keys in natural order, prefix [0, 1024*(m+1))),
which covers every core's visible range in that class. Causality inside the
budget is enforced by a host-provided additive mask (0 / -60000) that also
carries the diagonal triangle, so the program is core-independent while
skipping 37.5% of the score/AV matmul work.

Matmul precision: fp16 inputs (x, Wqk, Wov, attn) with fp32 PSUM
accumulation; q and o1 kept in fp16 on-chip. Softmax row max subtracted in
fp32; attn stored fp16 for the DMA-xbar transposes and AV.

Scheduling notes: input DMAs are issued in consumption order (xqt/wqk for
phase A first, then keys/masks); phase B runs slots largest-first so the
exp/transpose pipeline drains during B and phase E can start right after;
tiles are split per dependency unit (per-kg keys, per-chunk attn-transpose,
per-d o1) to keep cross-engine waits granular.
"""

import sys

sys.path.insert(0, "/opt/trn_rl_repo")

import numpy as np

import concourse.tile as tile
from concourse import bacc, mybir
from concourse.bass_utils import run_bass_kernel_spmd

T = 4096
D = 1024
NCORES = 8
RQ = T // NCORES  # 512 query rows per core
KC = D // 128  # 8 contraction chunks
NEG16 = -60000.0

BKT = [8, 16, 24, 32]  # key tiles (128) processed per slot
BG = [b // 4 for b in BKT]  # 512-wide key groups per slot
OFFK = [0, 1024, 3072, 6144]  # slot column offsets in ragged score layout
STOT = 10240  # total score/mask columns
MPOFF = [0, 2, 6, 12]  # mpart offsets (prefix of BG)
NCH = [b // 8 for b in BKT]  # 1024-wide exp chunks per slot: 1,2,3,4
LQOFF = [0, 1, 3, 6]  # lq offsets (prefix of NCH)

f32 = mybir.dt.float32
f16 = mybir.dt.float16


def _build_nc():
    nc = bacc.Bacc(
        "TRN2", target_bir_lowering=False, debug=False, num_devices=NCORES
    )

    xqt_d = nc.dram_tensor("xqt", [D, RQ], f16, kind="ExternalInput").ap()
    xtp_d = nc.dram_tensor("xtp", [D, T], f16, kind="ExternalInput").ap()
    xp_d = nc.dram_tensor("xp", [T, D], f16, kind="ExternalInput").ap()
    wqk_d = nc.dram_tensor("wqk", [D, D], f16, kind="ExternalInput").ap()
    wov_d = nc.dram_tensor("wov", [D, D], f16, kind="ExternalInput").ap()
    mask_d = nc.dram_tensor("mask", [128, STOT], f16, kind="ExternalInput").ap()
    out_d = nc.dram_tensor("out", [RQ, D], f32, kind="ExternalOutput").ap()

    with tile.TileContext(nc) as tc:
        # stack allocator: long-lived pools first
        consts = tc.alloc_tile_pool(name="consts", bufs=1)
        pt_pool = tc.alloc_tile_pool(name="ptpool", bufs=1)
        o1_pool = tc.alloc_tile_pool(name="o1pool", bufs=1)
        xpstream = tc.alloc_tile_pool(name="xpstream", bufs=3)
        p_pool = tc.alloc_tile_pool(name="ppool", bufs=3)
        s_pool = tc.alloc_tile_pool(name="spool", bufs=2)
        qt_pool = tc.alloc_tile_pool(name="qt", bufs=1)
        xtp_pool = tc.alloc_tile_pool(name="xtpp", bufs=1)
        mask_pool = tc.alloc_tile_pool(name="maskp", bufs=1)
        wqk_pool = tc.alloc_tile_pool(name="wqkp", bufs=1)
        xqt_pool = tc.alloc_tile_pool(name="xqtp", bufs=1)

        # stats scratch: negmax 0:4, lsum 4:8, recip 8:12, mpart 12:32, lq 32:42
        smalls = consts.tile([128, 48], f32, name="smalls")
        negmax = smalls[:, 0:4]
        lsum = smalls[:, 4:8]
        recip = smalls[:, 8:12]
        mpart = smalls[:, 12:32]
        lq = smalls[:, 32:42]

        # transposed-attn chunk tiles [keys-part, kt_local(8), 128 rows]
        ptc = [
            [
                pt_pool.tile([128, 1024], f16, name=f"pt{m}_{c}")
                for c in range(NCH[m])
            ]
            for m in range(4)
        ]
        o1t = [o1_pool.tile([128, RQ], f16, name=f"o1t{d}") for d in range(KC)]
        qt_sb = qt_pool.tile([128, KC * RQ], f16, name="qt_sb")
        xtp_t = [
            xtp_pool.tile([128, KC * 512], f16, name=f"xtp{kg}")
            for kg in range(T // 512)
        ]
        mask_t = [
            mask_pool.tile([128, BKT[m] * 128], f16, name=f"mask{m}")
            for m in range(4)
        ]
        wqk_t = [
            wqk_pool.tile([128, KC * 256], f16, name=f"wqk{md2}")
            for md2 in range(KC // 2)
        ]
        xqt_sb = xqt_pool.tile([128, KC * RQ], f16, name="xqt_sb")

        # ---- input DMAs, issued in consumption order ---------------------
        def load_wqk(md2):
            nc.sync.dma_start(
                wqk_t[md2].rearrange("p (kc n) -> p kc n", kc=KC),
                wqk_d[:, md2 * 256 : (md2 + 1) * 256].rearrange(
                    "(kc p) n -> p kc n", p=128
                ),
            )

        def load_xtp(kg):
            nc.sync.dma_start(
                xtp_t[kg].rearrange("p (kc n) -> p kc n", kc=KC),
                xtp_d[:, kg * 512 : (kg + 1) * 512].rearrange(
                    "(kc p) n -> p kc n", p=128
                ),
            )

        def load_mask(m):
            nc.sync.dma_start(
                mask_t[m], mask_d[:, OFFK[m] : OFFK[m] + BKT[m] * 128]
            )

        nc.sync.dma_start(
            xqt_sb.rearrange("p (kc n) -> p kc n", kc=KC),
            xqt_d.rearrange("(kc p) n -> p kc n", p=128),
        )
        load_wqk(0)
        load_wqk(1)
        load_mask(3)
        load_wqk(2)
        load_xtp(0)
        load_wqk(3)
        load_xtp(1)
        for kg in range(2, 8):
            load_xtp(kg)
        load_mask(2)
        load_mask(1)
        load_mask(0)

        # ---- Phase A: qT = (xq @ Wqk)^T  -> [D, RQ] fp16 -----------------
        with tc.tile_pool(name="psA", bufs=2, space="PSUM") as psA:
            for md2 in range(KC // 2):
                for h in range(2):
                    mtd = md2 * 2 + h
                    ps = psA.tile([128, RQ], f32, name="ps_qt")
                    for kc in range(KC):
                        nc.tensor.matmul(
                            ps[:],
                            wqk_t[md2][
                                :, kc * 256 + h * 128 : kc * 256 + h * 128 + 128
                            ],
                            xqt_sb[:, kc * RQ : (kc + 1) * RQ],
                            start=(kc == 0),
                            stop=(kc == KC - 1),
                        )
                    nc.vector.tensor_copy(
                        qt_sb[:, mtd * RQ : (mtd + 1) * RQ], ps[:]
                    )

        # ---- Phase B: per-slot scores + mask + softmax stats + exp/T -----
        # slots largest-first so the exp/transpose pipeline overlaps B
        with tc.tile_pool(name="psB", bufs=4, space="PSUM") as psB:
            for m in (3, 2, 1, 0):
                s_t = s_pool.tile([128, BKT[m] * 128], f32, name=f"s{m}", tag="s")
                for kg in range(BG[m]):
                    ps = psB.tile([128, 512], f32, name="ps_s", tag="psb")
                    for kc in range(KC):
                        nc.tensor.matmul(
                            ps[:],
                            qt_sb[:, kc * RQ + m * 128 : kc * RQ + (m + 1) * 128],
                            xtp_t[kg][:, kc * 512 : (kc + 1) * 512],
                            start=(kc == 0),
                            stop=(kc == KC - 1),
                        )
                    dst = s_t[:, kg * 512 : (kg + 1) * 512]
                    nc.vector.tensor_add(
                        dst, ps[:], mask_t[m][:, kg * 512 : (kg + 1) * 512]
                    )
                    nc.vector.tensor_reduce(
                        mpart[:, MPOFF[m] + kg : MPOFF[m] + kg + 1],
                        dst,
                        axis=mybir.AxisListType.X,
                        op=mybir.AluOpType.max,
                    )
                nc.vector.tensor_reduce(
                    negmax[:, m : m + 1],
                    mpart[:, MPOFF[m] : MPOFF[m] + BG[m]],
                    axis=mybir.AxisListType.X,
                    op=mybir.AluOpType.max,
                    negate=True,
                )
                # exp in 1024-wide chunks, transposed onto the ACT xbar ring
                for cidx in range(NCH[m]):
                    p_q = p_pool.tile([128, 1024], f16, name="p_q", tag="pq")
                    nc.scalar.activation(
                        p_q[:],
                        s_t[:, cidx * 1024 : (cidx + 1) * 1024],
                        mybir.ActivationFunctionType.Exp,
                        bias=negmax[:, m : m + 1],
                        scale=1.0,
                        accum_out=lq[:, LQOFF[m] + cidx : LQOFF[m] + cidx + 1],
                    )
                    nc.scalar.dma_start_transpose(
                        ptc[m][cidx].rearrange("p (kt r) -> p kt r", r=128),
                        p_q[:],
                    )
                nc.vector.tensor_reduce(
                    lsum[:, m : m + 1],
                    lq[:, LQOFF[m] : LQOFF[m] + NCH[m]],
                    axis=mybir.AxisListType.X,
                    op=mybir.AluOpType.add,
                )
                nc.vector.reciprocal(recip[:, m : m + 1], lsum[:, m : m + 1])

        xqt_pool.release()
        wqk_pool.release()
        mask_pool.release()
        xtp_pool.release()
        qt_pool.release()
        s_pool.release()

        # ---- Phase E: o1T[d] = sum_kt xp[kt,d]^T @ attn^T[kt] ------------
        wovstream = tc.alloc_tile_pool(name="wovstream", bufs=2)
        with tc.tile_pool(name="psE", bufs=1, space="PSUM") as psE_pool:
            psE = [
                psE_pool.tile([128, RQ], f32, name=f"psE{d}") for d in range(KC)
            ]
            # kts 8..31 first (need only slots 1-3, whose exps finish during
            # B since B runs slots largest-first); kts 0..7 last, by which
            # time slot0's post-B exp/transpose has landed. Removes the B->E
            # pipeline bubble. xp is loaded in 4-kt superchunks to stay
            # under the DMA ring-depth throttle.
            kt_chunks = [8, 12, 16, 20, 24, 28, 0, 4]
            for ci, kt0 in enumerate(kt_chunks):
                xp_t = xpstream.tile([128, 4 * D], f16, name="xp_t", tag="xp")
                nc.sync.dma_start(
                    xp_t.rearrange("p (four n) -> p four n", four=4),
                    xp_d[kt0 * 128 : (kt0 + 4) * 128, :].rearrange(
                        "(four p) n -> p four n", p=128
                    ),
                )
                for j in range(4):
                    kt = kt0 + j
                    for d in range(KC):
                        stat = xp_t[:, j * D + d * 128 : j * D + (d + 1) * 128]
                        for m in range(4):
                            if kt < BKT[m]:
                                # start_tensor_calc zeroes the WHOLE psum
                                # bank, so only the first matmul into bank d
                                # sets it; the other slot regions accumulate
                                # onto zeros. All chains end in the final kt
                                # block (0..7).
                                nc.tensor.matmul(
                                    psE[d][:, m * 128 : (m + 1) * 128],
                                    stat,
                                    ptc[m][kt // 8][
                                        :, (kt % 8) * 128 : (kt % 8 + 1) * 128
                                    ],
                                    start=(ci == 0 and j == 0 and m == 1),
                                    stop=(kt == 7),
                                    skip_group_check=True,
                                )
            # evacuate: split across DVE and Act so phase F starts sooner
            for d in range(KC):
                if d % 2 == 0:
                    nc.vector.tensor_copy(o1t[d][:], psE[d][:])
                else:
                    nc.scalar.activation(
                        o1t[d][:],
                        psE[d][:],
                        mybir.ActivationFunctionType.Copy,
                    )

        # ---- Phase F: out = (o1 @ Wov) * recip ---------------------------
        with (
            tc.tile_pool(name="psF", bufs=2, space="PSUM") as psF,
            tc.tile_pool(name="outp", bufs=3) as outp,
        ):
            for nb in range(2):
                wov_blk = wovstream.tile(
                    [128, KC * 512], f16, name="wov_blk", tag="wv"
                )
                nc.sync.dma_start(
                    wov_blk.rearrange("p (kc n) -> p kc n", kc=KC),
                    wov_d[:, nb * 512 : (nb + 1) * 512].rearrange(
                        "(kc p) n -> p kc n", p=128
                    ),
                )
                for m in range(4):
                    ps = psF.tile([128, 512], f32, name="ps_o")
                    for kc in range(KC):
                        nc.tensor.matmul(
                            ps[:],
                            o1t[kc][:, m * 128 : (m + 1) * 128],
                            wov_blk[:, kc * 512 : (kc + 1) * 512],
                            start=(kc == 0),
                            stop=(kc == KC - 1),
                        )
                    ob = outp.tile([128, 512], f32, name="ob")
                    nc.vector.tensor_scalar_mul(ob[:], ps[:], recip[:, m : m + 1])
                    nc.sync.dma_start(
                        out_d[m * 128 : (m + 1) * 128, nb * 512 : (nb + 1) * 512],
                        ob[:],
                    )

        wovstream.release()
        p_pool.release()
        xpstream.release()
        o1_pool.release()
        pt_pool.release()
        consts.release()

    nc.compile()
    return nc


_NC_CACHE = {}


def _get_nc():
    if "nc" not in _NC_CACHE:
        _NC_CACHE["nc"] = _build_nc()
    return _NC_CACHE["nc"]


def _slot_tiles(c):
    return [c, 15 - c, 16 + c, 31 - c]


def _prep_in_maps(x, Wqk, Wov):
    x = np.ascontiguousarray(np.asarray(x), dtype=np.float32)
    Wqk = np.ascontiguousarray(np.asarray(Wqk), dtype=np.float32)
    Wov = np.ascontiguousarray(np.asarray(Wov), dtype=np.float32)
    x16 = x.astype(np.float16)
    xT16 = np.ascontiguousarray(x16.T)  # [D, T]
    wqk16 = Wqk.astype(np.float16)
    wov16 = Wov.astype(np.float16)

    in_maps = []
    for c in range(NCORES):
        tiles = _slot_tiles(c)
        rows = np.concatenate(
            [np.arange(t * 128, (t + 1) * 128) for t in tiles]
        )
        xqt = np.ascontiguousarray(xT16[:, rows])
        mask = np.full((128, STOT), NEG16, dtype=np.float16)
        p = np.arange(128)[:, None]
        for m, t in enumerate(tiles):
            g = t * 128 + p  # global row index per partition
            y = np.arange(BKT[m] * 128)[None, :]  # global key index
            mask[:, OFFK[m] : OFFK[m] + BKT[m] * 128] = np.where(
                y <= g, np.float16(0.0), np.float16(NEG16)
            )
        in_maps.append(
            {
                "xqt": xqt,
                "xtp": xT16,
                "xp": x16,
                "wqk": wqk16,
                "wov": wov16,
                "mask": mask,
            }
        )
    return in_maps


def run(x, Wqk, Wov, **spmd_kwargs):
    """Full pipeline; returns (output [T, D] fp32, BassKernelResults)."""
    import time

    nc = _get_nc()
    in_maps = _prep_in_maps(x, Wqk, Wov)
    try:
        res = run_bass_kernel_spmd(
            nc, in_maps, core_ids=list(range(NCORES)), **spmd_kwargs
        )
    except Exception:
        # a prior crashed execution can leave a core transiently
        # unrecoverable; the runtime resets it — retry once
        time.sleep(10)
        res = run_bass_kernel_spmd(
            nc, in_maps, core_ids=list(range(NCORES)), **spmd_kwargs
        )
    out = np.empty((T, D), dtype=np.float32)
    for c in range(NCORES):
        co = res.results[c]["out"]
        for m, t in enumerate(_slot_tiles(c)):
            out[t * 128 : (t + 1) * 128] = co[m * 128 : (m + 1) * 128]
    return np.ascontiguousarray(out), res


def kernel(x, Wqk, Wov):
    out, _ = run(x, Wqk, Wov)
    return out
